# revision 1
# baseline (speedup 1.0000x reference)
"""Correlation layer kernel for 8 Trainium2 NeuronCores.

Data-parallel over batch: 16 samples -> 2 per core. Each core computes the
full 81-displacement correlation for its samples on-device via XLA/neuronx;
host concatenates the shards.
"""
import numpy as np
import jax
import jax.numpy as jnp

PAD = 4
MAX_DISP = 4
D = 2 * MAX_DISP + 1  # 9
N_CORES = 8


@jax.jit
def _corr(f1, f2):
    # f1, f2: [b, C, H, W] -> out [b, 81, H, W]
    H, W = f1.shape[2], f1.shape[3]
    f2p = jnp.pad(f2, ((0, 0), (0, 0), (PAD, PAD), (PAD, PAD)))
    outs = [
        jnp.mean(f1 * f2p[:, :, dy:dy + H, dx:dx + W], axis=1, keepdims=True)
        for dy in range(D)
        for dx in range(D)
    ]
    return jnp.concatenate(outs, axis=1)


def kernel(features1, features2):
    features1 = np.asarray(features1, dtype=np.float32)
    features2 = np.asarray(features2, dtype=np.float32)
    B = features1.shape[0]
    devs = jax.devices()[:N_CORES]
    n = min(N_CORES, B)
    shard = B // n

    futs = []
    for i in range(n):
        lo, hi = i * shard, (i + 1) * shard if i < n - 1 else B
        a = jax.device_put(features1[lo:hi], devs[i])
        b = jax.device_put(features2[lo:hi], devs[i])
        futs.append(_corr(a, b))
    out = np.concatenate([np.asarray(f) for f in futs], axis=0)
    return out.astype(np.float32)


if __name__ == "__main__":
    rng = np.random.default_rng(0)
    f1 = rng.standard_normal((16, 256, 64, 128), dtype=np.float32)
    f2 = rng.standard_normal((16, 256, 64, 128), dtype=np.float32)
    y = kernel(features1=f1, features2=f2)
    print("out shape:", y.shape, y.dtype)

